# revision 1
# baseline (speedup 1.0000x reference)
"""DigitCaps dynamic-routing kernel for 8 TRN2 NeuronCores.

Problem (hardcoded): x [256,1152,8] f32, W [1,1152,10,16,8] f32, 3 routing
iterations -> v [256,10,16,1] f32.

Strategy: shard the R=1152 routes 8-ways (144 per core), keep the full batch
B=256 on every core. u_hat is never materialized; each routing iteration
streams W through the TensorEngine:
  s_c[o,b]   = sum_{(r,i)} Ws_c[(r,i),o] * (en_c[r,b] * x[(r,i),b])   (PE)
  (AllReduce s over the 8 R-shards, squash -> v on every core)
  M_c[b,(r,i)] = sum_o v_c[b,o] * WoT_c[o,(r,i)]                      (PE)
  a_c[b,r]   = sum_i x[b,(r,i)] * M_c[b,(r,i)]                        (DVE)

Optimizations vs the original baseline (1113us -> ~384us HW):
  - W contractions (s0 / s-matmul / a-phase M-matmul) use float32r
    end-to-end (2 cycles/row fp32_mode=HIGH vs 4 for plain fp32; the
    producer must emit f32r dtype or the BIR verifier rejects it);
  - s / v live in [co, b] layout so the AllReduce bounce writes are
    contiguous (the old [b, co] transposed drain was a 4-byte-element
    scatter DMA costing up to 120us) and the a-phase reads v-slices
    directly (v_transpose deleted; final [b, co] transpose via PE once);
  - the en replicate-by-8 broadcast DMAs (90/iter, ~2.2us each of DMA
    queue time) are replaced by bf16 PE replication matmuls against a 0/1
    selector, with the en*x product read straight out of PSUM by the DVE;
  - en / its transposes run in bf16 (proven-safe: only the softmax
    weights are rounded, not W or x);
  - a-phase blocks are split between a DVE-from-PSUM path and an
    ACT-copy + GpSimd-multiply path to balance the three engines;
  - softmax max-shift is required (squash is elementwise so v saturates
    to +-1/element; logits reach ~+-70/round and exp would overflow);
    broadcasts use step-0 APs, reciprocals use reciprocal_approx_fast;
  - the AllReduce is split per capsule-group (c 0..7 / 8..9) so squash,
    v-staging and the first a-phase blocks overlap the second collective;
    dummy keepalive matmul chains cover the collective windows to keep
    the PE HAM activity monitor from dropping to the 1.2GHz cold clock.
W/x stay f32(r): the routing argmax is chaotic under bf16 W/x rounding
(measured 5e-2 output error vs 2e-2 tolerance).
"""

import sys

if "/opt/trn_rl_repo" not in sys.path:
    sys.path.insert(0, "/opt/trn_rl_repo")

import numpy as np

import concourse.bass as bass
import concourse.tile as tile
from concourse import bacc, mybir
from concourse.bass_utils import run_bass_kernel_spmd
from concourse.masks import make_identity

F32 = mybir.dt.float32
F32R = mybir.dt.float32r
BF16 = mybir.dt.bfloat16

NCORES = 8
B, R, C, O, I = 256, 1152, 10, 16, 8
RL = R // NCORES          # 144 routes per core
RI = RL * I               # 1152 (r,i) rows per core
NT = RI // 128            # 9 K-chunks of 128
CO = C * O                # 160
BH = B // 128             # 2 batch half-tiles
HA = RI // 3              # 384-wide a-phase chunks

AP = bass.AP


def _insert_bcast(base, pos, count):
    """Insert a step-0 (broadcast) free dim into an existing AP at index pos."""
    dims = list(base.ap)
    dims.insert(pos, [0, count])
    return AP(tensor=base.tensor, offset=base.offset, ap=dims)


def build_kernel(n_iters: int, reps: int = 1, collectives: bool = True):
    nc = bacc.Bacc("TRN2", target_bir_lowering=False, debug=False,
                   num_devices=NCORES)

    xt_in = nc.dram_tensor("xt", [128, NT, B], F32R, kind="ExternalInput")
    xb_in = nc.dram_tensor("xb", [128, BH, RI], F32, kind="ExternalInput")
    ws = nc.dram_tensor("ws", [128, NT, CO], F32R, kind="ExternalInput")
    wot = nc.dram_tensor("wot", [16, C, RI], F32R, kind="ExternalInput")
    rep_in = nc.dram_tensor("rep", [128, 8, 128], BF16, kind="ExternalInput")
    rep2_in = nc.dram_tensor("rep2", [16, 128], BF16, kind="ExternalInput")
    out = nc.dram_tensor("out", [B, CO], F32, kind="ExternalOutput")

    with tile.TileContext(nc) as tc:
        with (
            tc.tile_pool(name="stat", bufs=1) as stat,
            tc.tile_pool(name="work", bufs=2) as work,
            tc.tile_pool(name="sm", bufs=1) as smp,
            tc.tile_pool(name="mtp", bufs=3) as mtp,
            tc.tile_pool(name="ytp", bufs=2) as ytp,
            tc.tile_pool(name="dram", bufs=2, space="DRAM") as dram,
            tc.tile_pool(name="ps_mp", bufs=2, space="PSUM") as ps_mp,
            tc.tile_pool(name="ps_yp", bufs=2, space="PSUM") as ps_yp,
            tc.tile_pool(name="ps_ep", bufs=2, space="PSUM") as ps_ep,
            tc.tile_pool(name="ps_sp", bufs=2, space="PSUM") as ps_sp,
        ):
            # ---- static SBUF tensors ----
            XT = stat.tile([128, NT, B], F32R)        # x^T  [(r,i)%128, t, b]
            XB = stat.tile([128, BH, RI], F32)       # x    [b%128, bh, (r,i)]
            WS = stat.tile([128, NT, CO], F32R)       # W as lhsT for s-matmul
            WOT = stat.tile([16, C, RI], F32R)        # W^T as rhs for M-matmul
            REP = stat.tile([128, 8, 128], BF16)      # replicate-by-8 selectors
            REP2 = stat.tile([16, 128], BF16)         # chunk t=8 selector
            IDENT = stat.tile([128, 128], F32)
            IDENTB = stat.tile([128, 128], BF16)
            nc.sync.dma_start(out=XT, in_=xt_in[:])
            nc.sync.dma_start(out=WS, in_=ws[:])
            nc.scalar.dma_start(out=XB, in_=xb_in[:])
            nc.scalar.dma_start(out=WOT, in_=wot[:])
            nc.scalar.dma_start(out=REP, in_=rep_in[:])
            nc.scalar.dma_start(out=REP2, in_=rep2_in[:])
            make_identity(nc, IDENT[:, :])
            make_identity(nc, IDENTB[:, :])

            # logits b_ij, layout [p=b%128, (bh, c, r)]
            blog = stat.tile([128, BH, C, RL], F32)

            # v (squashed capsule outputs), [co, b] layout, co split 128+32.
            # f32 copy feeds the final output transpose; f32r copy feeds the
            # a-phase matmul (f32/f32r operands cannot be mixed).
            v1 = stat.tile([128, B], F32)
            v2 = stat.tile([32, B], F32)
            vr1 = stat.tile([128, B], F32R)
            vr2 = stat.tile([32, B], F32R)
            # vrt[o, c, b]: a-phase lhsT must start at partition 0/32/64,
            # so v-slices are re-staged per capsule via small SBUF DMAs.
            vrt = stat.tile([16, C, B], F32R)

            def stage_vrt(cs):
                for c in cs:
                    src = (vr1[16 * c:16 * (c + 1), :] if c < 8
                           else vr2[16 * (c - 8):16 * (c - 7), :])
                    qeng = nc.sync if (c % 2 == 0) else nc.scalar
                    qeng.dma_start(out=vrt[:, c, :], in_=src)

            def s0_matmul():
                """s0 = sum_r u_hat  ->  psum [co, b] (two tiles)."""
                p1 = ps_ep.tile([128, B], F32, tag="ep")
                p2 = ps_sp.tile([32, B], F32, tag="sp")
                for t in range(NT):
                    nc.tensor.matmul(p1, WS[:, t, 0:128], XT[:, t, :],
                                     start=(t == 0), stop=(t == NT - 1))
                for t in range(NT):
                    nc.tensor.matmul(p2, WS[:, t, 128:160], XT[:, t, :],
                                     start=(t == 0), stop=(t == NT - 1))
                return [(p1, 128, 0)], [(p2, 32, 0)]

            def allreduce_g(writes, grp):
                """One c-group's AllReduce. writes: (ptile, nrows, row0).
                grp 0 covers co 0..128 (c 0..7), grp 1 covers co 128..160.
                Returns (bounce_out, last_drain_tile)."""
                nr = 128 if grp == 0 else 32
                b_in = dram.tile([nr, B], F32, tag=f"ari{grp}")
                b_out = dram.tile([nr, B], F32, tag=f"aro{grp}")
                last_sb = None
                for ptile, nrows, r0 in writes:
                    sb = work.tile([nrows, B], F32, tag=f"sd{grp}_{nrows}")
                    nc.scalar.copy(sb[:, :], ptile[0:nrows, :])
                    nc.sync.dma_start(out=b_in[r0:r0 + nrows, :], in_=sb)
                    last_sb = sb
                if collectives:
                    nc.gpsimd.collective_compute(
                        "AllReduce",
                        mybir.AluOpType.add,
                        replica_groups=[list(range(NCORES))],
                        ins=[b_in[:].opt()],
                        outs=[b_out[:].opt()],
                    )
                else:
                    nc.sync.dma_start(out=b_out[:], in_=b_in[:])
                return b_out, last_sb

            def keepalive(seed, n, nb=128, lhs=None):
                """Chain of dummy accumulating matmuls anchored on `seed`:
                holds the PE HAM activity window open across an engine-idle
                stretch (AllReduce / softmax) so later matmuls run at 2.4GHz
                instead of the 1.2GHz cold clock. Result is never read."""
                kp = ps_sp.tile([16, nb], F32, tag="sp")
                li = lhs if lhs is not None else IDENT
                for i in range(n):
                    nc.tensor.matmul(kp, li[0:16, 0:16], seed,
                                     start=(i == 0), stop=(i == n - 1))

            def squash_g(b_out, scale, grp, stage=True):
                """load s [rows,b] from bounce, v = s*|s|/(1+s^2) (s*=scale)."""
                s, v, vr, nr = ((None, v1, vr1, 128) if grp == 0
                                else (None, v2, vr2, 32))
                s = work.tile([nr, B], F32, tag=f"sq_s{nr}")
                nc.sync.dma_start(out=s, in_=b_out[0:nr, :])
                sf = s[:, :]
                sq = work.tile([nr, B], F32, tag=f"sq_sq{nr}")
                ab = work.tile([nr, B], F32, tag=f"sq_ab{nr}")
                den = work.tile([nr, B], F32, tag=f"sq_den{nr}")
                if scale != 1.0:
                    nc.scalar.mul(sf, sf, scale)
                nc.scalar.square(sq[:, :], sf)
                nc.scalar.sqrt(ab[:, :], sq[:, :])
                nc.vector.tensor_scalar_add(den[:, :], sq[:, :], 1.0)
                nc.vector.reciprocal_approx_fast(den[:, :], den[:, :])
                nc.vector.tensor_mul(ab[:, :], ab[:, :], den[:, :])
                nc.vector.tensor_mul(v[:, :], ab[:, :], sf)
                if stage:
                    nc.scalar.copy(vr[:, :], v[:, :])

            def a_blocks(cs, dst):
                """dst[.,bh,c,.] = sum_i x*M, M = v_c @ WoT_c (capsule group).
                dst is blog itself on the first iteration (no separate
                accumulator / copy), or a fresh ar tile afterwards."""
                for c in cs:
                    for bh in range(BH):
                        lhs = vrt[:, c, bh * 128:(bh + 1) * 128]
                        mt = mtp.tile([128, RI], F32, tag="mtmp")
                        if bh == 0 and c % 3 != 0:
                            # DVE multiplies straight out of PSUM
                            for h in range(3):
                                mp = ps_mp.tile([128, HA], F32, tag="mpsum")
                                nc.tensor.matmul(
                                    mp[:, :], lhs,
                                    WOT[:, c, h * HA:(h + 1) * HA],
                                    start=True, stop=True)
                                nc.vector.tensor_mul(
                                    mt[:, h * HA:(h + 1) * HA], mp[:, :],
                                    XB[:, bh, h * HA:(h + 1) * HA])
                        else:
                            # ACT drains PSUM, GpSimd multiplies (keeps the
                            # DVE free: it is the binding engine here)
                            ms = mtp.tile([128, RI], F32, tag="mstage")
                            for h in range(3):
                                mp = ps_mp.tile([128, HA], F32, tag="mpsum")
                                nc.tensor.matmul(
                                    mp[:, :], lhs,
                                    WOT[:, c, h * HA:(h + 1) * HA],
                                    start=True, stop=True)
                                nc.scalar.copy(
                                    ms[:, h * HA:(h + 1) * HA], mp[:, :])
                            nc.gpsimd.tensor_mul(mt[:, :], ms[:, :],
                                                 XB[:, bh, :])
                        tv = mt[:, :].rearrange("p (r i) -> p r i", i=I)
                        nc.vector.tensor_reduce(dst[:, bh, c, :], tv,
                                                axis=mybir.AxisListType.X,
                                                op=mybir.AluOpType.add)
            def blog_update(ar):
                # split per batch-half so the softmax chain for bh=0 can
                # start while bh=1 is still accumulating
                for bh in range(BH):
                    nc.vector.tensor_add(blog[:, bh, :, :], blog[:, bh, :, :],
                                         ar[:, bh, :, :])

            def s_phase():
                """softmax(blog) -> en -> enT (PE) -> rep (PE) -> y -> s."""
                # logits reach ~±70 per routing round (squash is elementwise,
                # so v saturates to +-1 per element): exp needs max-shift.
                # Chain is pipelined per batch-half: bh=0's transposes can
                # start while bh=1 is still in the DVE softmax chain.
                mx = smp.tile([128, BH, RL], F32, tag="mx")
                e = smp.tile([128, BH, C, RL], F32, tag="e")
                z = smp.tile([128, BH, RL], F32, tag="z")
                for bh in range(BH):
                    bv = blog[:, bh, :, :].rearrange("p c r -> p r c")
                    nc.vector.tensor_reduce(mx[:, bh, :], bv,
                                            axis=mybir.AxisListType.X,
                                            op=mybir.AluOpType.max)
                    nc.vector.tensor_sub(e[:, bh, :, :], blog[:, bh, :, :],
                                         _insert_bcast(mx[:, bh, :], 1, C))
                    nc.scalar.activation(e[:, bh, :, :], e[:, bh, :, :],
                                         mybir.ActivationFunctionType.Exp)
                    ev = e[:, bh, :, :].rearrange("p c r -> p r c")
                    nc.vector.tensor_reduce(z[:, bh, :], ev,
                                            axis=mybir.AxisListType.X,
                                            op=mybir.AluOpType.add)
                    nc.vector.reciprocal_approx_fast(z[:, bh, :],
                                                     z[:, bh, :])
                en = smp.tile([128, BH, C, RL], BF16, tag="en")

                writes = []
                for c in range(C):
                    for bh in range(BH):
                        nc.vector.tensor_mul(en[:, bh, c, :],
                                             e[:, bh, c, :], z[:, bh, :])
                    # transpose en_c -> [r, b]: both r-chunks share one
                    # bf16 psum bank so the pool double-buffers across c
                    ept = ps_ep.tile([128, 2 * B], BF16, tag="ep")
                    for bh in range(BH):
                        bs = slice(bh * 128, (bh + 1) * 128)
                        nc.tensor.matmul(ept[:, bs], en[:, bh, c, 0:128],
                                         IDENTB[:, :], start=True, stop=True,
                                         is_transpose=True)
                        nc.tensor.matmul(ept[0:16, B + bh * 128:
                                             B + (bh + 1) * 128],
                                         en[:, bh, c, 128:RL],
                                         IDENTB[:, :], start=True, stop=True,
                                         is_transpose=True)
                    et1 = work.tile([128, B], BF16, tag="et1")
                    et2 = work.tile([16, B], BF16, tag="et2")
                    nc.scalar.copy(et1[:, :], ept[:, 0:B])
                    nc.scalar.copy(et2[:, :], ept[0:16, B:2 * B])

                    # replicate r->(r,i) on the PE, y = en_rep * x from PSUM
                    ytc = ytp.tile([128, NT, B], F32R, tag="ytc")
                    for pr in range(5):
                        t0 = 2 * pr
                        nn = 1 if pr == 4 else 2
                        yp = ps_yp.tile([128, 2 * B], F32, tag="yp")
                        for k in range(nn):
                            t = t0 + k
                            dst = yp[:, k * B:(k + 1) * B]
                            if t < 8:
                                nc.tensor.matmul(dst, REP[:, t, :],
                                                 et1[:, :],
                                                 start=True, stop=True)
                            else:
                                nc.tensor.matmul(dst, REP2[:, :],
                                                 et2[:, :],
                                                 start=True, stop=True)
                        nc.vector.tensor_mul(
                            ytc[:, t0:t0 + nn, :], yp[:, 0:nn * B],
                            XT[:, t0:t0 + nn, :])

                    sp = ps_sp.tile([16, B], F32, tag="sp")
                    for t in range(NT):
                        nc.tensor.matmul(sp, WS[:, t, c * 16:(c + 1) * 16],
                                         ytc[:, t, :],
                                         start=(t == 0), stop=(t == NT - 1))
                    writes.append((sp, 16, (c % 8) * 16))
                return writes[0:8], writes[8:10]

            # ---------------- routing ----------------
            for _rep in range(reps):
                wA, wB = s0_matmul()
                scale = 0.1
                for it in range(1, n_iters):
                    boA, sdA = allreduce_g(wA, 0)
                    boB, _ = allreduce_g(wB, 1)
                    keepalive(sdA[0:16, 0:128], 20)
                    dst = (blog if it == 1
                           else smp.tile([128, BH, C, RL], F32, tag="ared"))
                    squash_g(boA, scale, 0)
                    stage_vrt(range(0, 8))
                    a_blocks(range(0, 8), dst)
                    squash_g(boB, scale, 1)
                    stage_vrt(range(8, C))
                    a_blocks(range(8, C), dst)
                    if it != 1:
                        blog_update(dst)
                    scale = 1.0
                    wA, wB = s_phase()
                # final AllReduce + squash -> v
                boA, sdA = allreduce_g(wA, 0)
                boB, _ = allreduce_g(wB, 1)
                keepalive(sdA[0:16, 0:128], 10)
                squash_g(boA, scale, 0, stage=False)
                squash_g(boB, scale, 1, stage=False)

            # ---- final transpose v [co,b] -> vout [b%128, bh, co], DMA out
            vout = stat.tile([128, BH, CO], F32)
            for bh in range(BH):
                bs = slice(bh * 128, (bh + 1) * 128)
                tp1 = ps_ep.tile([128, 128], F32, tag="ep")
                nc.tensor.matmul(tp1, v1[:, bs], IDENT[:, :],
                                 start=True, stop=True, is_transpose=True)
                nc.scalar.copy(vout[:, bh, 0:128], tp1[:, :])
                tp2 = ps_ep.tile([128, 32], F32, tag="ep")
                nc.tensor.matmul(tp2, v2[:, bs], IDENT[0:32, 0:32],
                                 start=True, stop=True, is_transpose=True)
                nc.scalar.copy(vout[:, bh, 128:160], tp2[:, :])

            dst = out[:].rearrange("(bh p) co -> p bh co", p=128)
            nc.sync.dma_start(out=dst, in_=vout[:, :, :])

    nc.compile()
    return nc


def prep_inputs(x: np.ndarray, W: np.ndarray):
    """Host-side layout prep. Returns per-core input dicts."""
    W = W[0]  # [R, C, O, I]
    # replicate-by-8 selector masks (shared across cores)
    from ml_dtypes import bfloat16
    rep = np.zeros((128, 8, 128), dtype=bfloat16)
    for t in range(8):
        for j in range(128):
            rep[16 * t + j // 8, t, j] = 1.0
    rep2 = np.zeros((16, 128), dtype=bfloat16)
    for j in range(128):
        rep2[j // 8, j] = 1.0
    in_maps = []
    for k in range(NCORES):
        rs = slice(k * RL, (k + 1) * RL)
        xk = np.ascontiguousarray(x[:, rs, :])      # [B, RL, I]
        wk = np.ascontiguousarray(W[rs])            # [RL, C, O, I]
        xt = np.transpose(xk, (1, 2, 0)).reshape(NT, 128, B)
        xt = np.transpose(xt, (1, 0, 2))            # [128, NT, B]
        xb = xk.reshape(BH, 128, RI)
        xb = np.transpose(xb, (1, 0, 2))            # [128, BH, RI]
        # ws[p, t, c*16+o] = W[16t + p//8, c, o, p%8]
        wsk = np.transpose(wk.reshape(NT, 16, C, O, I), (0, 1, 4, 2, 3))
        wsk = wsk.reshape(NT, 128, CO)
        wsk = np.transpose(wsk, (1, 0, 2))          # [128, NT, CO]
        # wot[o, c, r*8+i] = W[r, c, o, i]
        wotk = np.transpose(wk, (2, 1, 0, 3)).reshape(O, C, RI)
        f32 = np.float32
        in_maps.append({
            "xt": np.ascontiguousarray(xt).astype(f32),
            "xb": np.ascontiguousarray(xb).astype(f32),
            "ws": np.ascontiguousarray(wsk).astype(f32),
            "wot": np.ascontiguousarray(wotk).astype(f32),
            "rep": rep,
            "rep2": rep2,
        })
    return in_maps


_CACHE = {}


def _get_nc(n_iters: int):
    if n_iters not in _CACHE:
        _CACHE[n_iters] = build_kernel(n_iters)
    return _CACHE[n_iters]


def kernel(x, W, num_iterations, _trace=False):
    n = int(num_iterations)
    assert n >= 1
    nc = _get_nc(n)
    in_maps = prep_inputs(np.asarray(x, dtype=np.float32),
                          np.asarray(W, dtype=np.float32))
    res = run_bass_kernel_spmd(nc, in_maps, list(range(NCORES)),
                               trace=_trace)
    v = res.results[0]["out"].reshape(B, C, O, 1).astype(np.float32)
    kernel.last_results = res
    return v

